# revision 5
# baseline (speedup 1.0000x reference)
"""Trainium2 Bass kernel for grouped per-block linear:
    y[b, g] = sum_d x[b, g*6+d] * W[g, d] + b[g]
x: [4194304, 60] f32 -> y: [4194304, 10] f32

Strategy (pure data parallel, 8 cores):
  - shard x by batch into 8 contiguous row blocks of 524288 rows
  - per core: tiles of [128 partitions, T rows/partition], partition-major
    rows (partition p owns T consecutive rows) so every DMA is
    per-partition-contiguous in DRAM.
  - dense in-place elementwise multiply x *= broadcast(W) split across the
    DVE and GPSIMD engines, grouped DVE tensor_reduce over the innermost
    6, then a DVE bias add against a host-broadcast bias tile.
  - memory-bound target: ~147 MB DMA traffic per core (~410 us at
    358 GB/s per-NC HBM bandwidth).
"""

import numpy as np

# ---------------- hardcoded problem constants ----------------
B_TOTAL = 4_194_304
N_CORES = 8
R = B_TOTAL // N_CORES  # 524288 rows per core
G = 10                  # groups
D = 6                   # group dim
DW = G * D              # 60 features per row
P = 128                 # partitions
T = 64                  # rows per partition per tile
TILE_ROWS = P * T       # 8192 rows per tile
N_TILES = R // TILE_ROWS  # 64 iterations

# Iterations whose multiply runs on GPSIMD (Pool engine); DVE does all
# reduces + bias adds + remaining muls.  GPSIMD TT is ~2x the DVE cost
# per element, so ~46/64 on GPSIMD balances the two engines ~equally.
_N_GP = 46
_GP_SET = frozenset(
    int(round(i * N_TILES / _N_GP)) for i in range(_N_GP)
)

_CACHE = {}


def _build_bass():
    import concourse.bacc as bacc
    import concourse.mybir as mybir
    import concourse.tile as tile

    f32 = mybir.dt.float32
    nc = bacc.Bacc("TRN2", target_bir_lowering=False, debug=False)

    xs = nc.dram_tensor("xs", [R, DW], f32, kind="ExternalInput")
    wbc = nc.dram_tensor("wbc", [P, T * DW], f32, kind="ExternalInput")
    bbc = nc.dram_tensor("bbc", [P, T * G], f32, kind="ExternalInput")
    ys = nc.dram_tensor("ys", [R, G], f32, kind="ExternalOutput")

    xs_r = xs[:, :].rearrange("(n p t) d -> n p (t d)", p=P, t=T)
    ys_r = ys[:, :].rearrange("(n p t) g -> n p (t g)", p=P, t=T)

    with tile.TileContext(nc) as tc:
        with (
            tc.tile_pool(name="consts", bufs=1) as cpool,
            tc.tile_pool(name="xin", bufs=5) as xpool,
            tc.tile_pool(name="yout", bufs=4) as ypool,
        ):
            wt = cpool.tile([P, T * DW], f32, tag="wbc")
            nc.sync.dma_start(wt, wbc[:, :])
            bt = cpool.tile([P, T * G], f32, tag="bbc")
            nc.sync.dma_start(bt, bbc[:, :])

            for i in range(N_TILES):
                xt = xpool.tile([P, T * DW], f32, tag="x")
                nc.sync.dma_start(xt, xs_r[i])

                eng = nc.gpsimd if i in _GP_SET else nc.vector
                eng.tensor_tensor(xt, xt, wt, mybir.AluOpType.mult)

                yt = ypool.tile([P, T * G], f32, tag="y")
                xt3 = xt.rearrange("p (tg d) -> p tg d", d=D)
                nc.vector.tensor_reduce(
                    yt, xt3, mybir.AxisListType.X, mybir.AluOpType.add
                )
                nc.vector.tensor_tensor(yt, yt, bt, mybir.AluOpType.add)
                nc.scalar.dma_start(ys_r[i], yt)

    nc.compile()
    return nc


def _get_bass():
    if "nc" not in _CACHE:
        _CACHE["nc"] = _build_bass()
    return _CACHE["nc"]


def _host_consts(W, b):
    # wbc[p, t*60 + g*6 + d] = W[g, d]
    wflat = np.ascontiguousarray(W, dtype=np.float32).reshape(DW)
    wbc = np.tile(wflat, (P, T)).astype(np.float32)
    # bbc[p, t*10 + g] = b[g]
    bflat = np.asarray(b, dtype=np.float32).reshape(G)
    bbc = np.tile(bflat, (P, T)).astype(np.float32)
    return np.ascontiguousarray(wbc), np.ascontiguousarray(bbc)


def _run(x, W, b, **spmd_kwargs):
    from concourse import bass_utils

    x = np.ascontiguousarray(x, dtype=np.float32)
    assert x.shape == (B_TOTAL, DW), x.shape
    wbc, bbc = _host_consts(W, b)

    nc = _get_bass()
    in_maps = []
    for c in range(N_CORES):
        shard = x[c * R : (c + 1) * R]
        in_maps.append({"xs": shard, "wbc": wbc, "bbc": bbc})

    res = bass_utils.run_bass_kernel_spmd(
        nc, in_maps, core_ids=list(range(N_CORES)), **spmd_kwargs
    )
    y = np.concatenate([r["ys"] for r in res.results], axis=0)
    return y, res


def kernel(x, W, b):
    return _run(x, W, b)[0]


# revision 6
# speedup vs baseline: 1.1055x; 1.1055x over previous
"""Trainium2 Bass kernel for grouped per-block linear:
    y[b, g] = sum_d x[b, g*6+d] * W[g, d] + b[g]
x: [4194304, 60] f32 -> y: [4194304, 10] f32

Strategy (pure data parallel, 8 cores):
  - shard x by batch into 8 contiguous row blocks of 524288 rows
  - per core: tiles of [128 partitions, T rows/partition], partition-major
    rows (partition p owns T consecutive rows) so every DMA is
    per-partition-contiguous in DRAM.
  - elementwise multiply x * broadcast(W), written strided (6-of-7) into a
    [T,10,7]-layout tmp whose 7th column holds the bias; a DVE
    tensor_reduce over the last axis of 7 then yields y + bias in one op.
  - The multiply is split between DVE and GPSIMD.  Key hardware fact: the
    DVE's 2nd read port and GPSIMD share ONE exclusive SBUF port pair, so
    DVE tensor_tensor (2 reads) fully serializes against GPSIMD work,
    while DVE tensor_reduce (1 read) runs concurrently.  The split is
    chosen so  GP_busy + DVE_TT_busy  ~=  DVE_total ~= DMA time.
  - memory-bound target: ~147 MB DMA traffic per core (~410 us at
    358 GB/s per-NC HBM bandwidth).
"""

import numpy as np

# ---------------- hardcoded problem constants ----------------
B_TOTAL = 4_194_304
N_CORES = 8
R = B_TOTAL // N_CORES  # 524288 rows per core
G = 10                  # groups
D = 6                   # group dim
DW = G * D              # 60 features per row
W7 = G * (D + 1)        # 70 = tmp row width (6 data + 1 bias col per group)
P = 128                 # partitions
T = 64                  # rows per partition per tile
TILE_ROWS = P * T       # 8192 rows per tile
N_TILES = R // TILE_ROWS  # 64 iterations
NTMP = 4                # rotating bias-initialized tmp buffers

# Iterations whose multiply runs on GPSIMD (Pool engine).  DVE does all
# reduces (no shared-port conflict) plus the remaining muls (which do
# conflict with GPSIMD).  Balance: DVE = 64*4.67 + n_dv*4.2,
# conflict-class = n_gp*9.9 + n_dv*4.2 -> n_gp ~= 30.
_N_GP = 30
_GP_SET = frozenset(
    int(round(i * N_TILES / _N_GP)) for i in range(_N_GP)
)

_CACHE = {}


def _build_bass():
    import concourse.bacc as bacc
    import concourse.mybir as mybir
    import concourse.tile as tile

    f32 = mybir.dt.float32
    nc = bacc.Bacc("TRN2", target_bir_lowering=False, debug=False)

    xs = nc.dram_tensor("xs", [R, DW], f32, kind="ExternalInput")
    wbc = nc.dram_tensor("wbc", [P, T * DW], f32, kind="ExternalInput")
    binit = nc.dram_tensor("binit", [P, T * W7], f32, kind="ExternalInput")
    ys = nc.dram_tensor("ys", [R, G], f32, kind="ExternalOutput")

    xs_r = xs[:, :].rearrange("(n p t) d -> n p (t d)", p=P, t=T)
    ys_r = ys[:, :].rearrange("(n p t) g -> n p (t g)", p=P, t=T)

    with tile.TileContext(nc) as tc:
        with (
            tc.tile_pool(name="consts", bufs=1) as cpool,
            tc.tile_pool(name="xin", bufs=4) as xpool,
            tc.tile_pool(name="tmps", bufs=1) as tpool,
            tc.tile_pool(name="yout", bufs=4) as ypool,
        ):
            wt = cpool.tile([P, T * DW], f32, tag="wbc")
            nc.sync.dma_start(wt, wbc[:, :])
            wt4 = wt.rearrange("p (t g d) -> p t g d", t=T, g=G, d=D)

            # Persistent tmp buffers: 7th column of each group pre-filled
            # with the bias; the muls only ever write columns 0..5.
            tmps = []
            for k in range(NTMP):
                tk = tpool.tile([P, T * W7], f32, tag=f"tmp{k}")
                nc.sync.dma_start(tk, binit[:, :])
                tmps.append(tk)

            for i in range(N_TILES):
                xt = xpool.tile([P, T * DW], f32, tag="x")
                nc.sync.dma_start(xt, xs_r[i])
                xt4 = xt.rearrange("p (t g d) -> p t g d", t=T, g=G, d=D)

                tmp = tmps[i % NTMP]
                tmp4 = tmp.rearrange("p (t g j) -> p t g j", t=T, g=G, j=D + 1)
                mul_out = tmp4[:, :, :, 0:D]
                eng = nc.gpsimd if i in _GP_SET else nc.vector
                eng.tensor_tensor(mul_out, xt4, wt4, mybir.AluOpType.mult)

                yt = ypool.tile([P, T * G], f32, tag="y")
                tmp3 = tmp.rearrange("p (tg j) -> p tg j", j=D + 1)
                nc.vector.tensor_reduce(
                    yt, tmp3, mybir.AxisListType.X, mybir.AluOpType.add
                )
                nc.scalar.dma_start(ys_r[i], yt)

    nc.compile()
    return nc


def _get_bass():
    if "nc" not in _CACHE:
        _CACHE["nc"] = _build_bass()
    return _CACHE["nc"]


def _host_consts(W, b):
    # wbc[p, t*60 + g*6 + d] = W[g, d]
    wflat = np.ascontiguousarray(W, dtype=np.float32).reshape(DW)
    wbc = np.tile(wflat, (P, T)).astype(np.float32)
    # binit[p, t*70 + g*7 + j] = b[g] if j == 6 else 0
    brow = np.zeros((G, D + 1), dtype=np.float32)
    brow[:, D] = np.asarray(b, dtype=np.float32)
    binit = np.tile(brow.reshape(W7), (P, T)).astype(np.float32)
    return np.ascontiguousarray(wbc), np.ascontiguousarray(binit)


def _run(x, W, b, **spmd_kwargs):
    from concourse import bass_utils

    x = np.ascontiguousarray(x, dtype=np.float32)
    assert x.shape == (B_TOTAL, DW), x.shape
    wbc, binit = _host_consts(W, b)

    nc = _get_bass()
    in_maps = []
    for c in range(N_CORES):
        shard = x[c * R : (c + 1) * R]
        in_maps.append({"xs": shard, "wbc": wbc, "binit": binit})

    res = bass_utils.run_bass_kernel_spmd(
        nc, in_maps, core_ids=list(range(N_CORES)), **spmd_kwargs
    )
    y = np.concatenate([r["ys"] for r in res.results], axis=0)
    return y, res


def kernel(x, W, b):
    return _run(x, W, b)[0]


# revision 9
# speedup vs baseline: 1.4318x; 1.2951x over previous
"""Trainium2 Bass kernel for grouped per-block linear:
    y[b, g] = sum_d x[b, g*6+d] * W[g, d] + b[g]
x: [4194304, 60] f32 -> y: [4194304, 10] f32

Strategy (pure data parallel, 8 cores):
  - shard x by batch into 8 contiguous row blocks of 524288 rows
  - per core: tiles of [128 partitions, T rows/partition], partition-major
    rows (partition p owns T consecutive rows) so every DMA is
    per-partition-contiguous in DRAM.
  - elementwise multiply x * broadcast(W), written strided (6-of-7) into a
    [T,10,7]-layout tmp whose 7th column holds the bias; a DVE
    tensor_reduce over the last axis of 7 then yields y + bias in one op.
  - The multiply is split between DVE and GPSIMD.  Key hardware fact: the
    DVE's 2nd read port and GPSIMD share ONE exclusive SBUF port pair, so
    DVE tensor_tensor (2 reads) fully serializes against GPSIMD work,
    while DVE tensor_reduce (1 read) runs concurrently.  The split is
    chosen so  GP_busy + DVE_TT_busy  ~=  DVE_total ~= DMA time.
  - memory-bound target: ~147 MB DMA traffic per core (~410 us at
    358 GB/s per-NC HBM bandwidth).
"""

import numpy as np

# ---------------- hardcoded problem constants ----------------
B_TOTAL = 4_194_304
N_CORES = 8
R = B_TOTAL // N_CORES  # 524288 rows per core
G = 10                  # groups
D = 6                   # group dim
DW = G * D              # 60 features per row
W7 = G * (D + 1)        # 70 = tmp row width (6 data + 1 bias col per group)
P = 128                 # partitions
T = 64                  # rows per partition per tile
TILE_ROWS = P * T       # 8192 rows per tile
N_TILES = R // TILE_ROWS  # 64 iterations
NTMP = 4                # rotating bias-initialized tmp buffers

# Iterations whose multiply runs on GPSIMD (Pool engine).  DVE does all
# reduces plus the remaining muls.  The DVE muls read the weights from a
# PSUM copy: a tensor_tensor with one PSUM operand uses the DVE's PSUM
# port + dedicated SBUF port only, so it does NOT take the shared
# DVE/GPSIMD SBUF port pair and runs fully concurrent with GPSIMD.
# Balance: DVE = 64*4.81 + n_dv*4.2 vs GP = n_gp*9.81 -> n_gp ~= 40.
_N_GP = 40
_GP_SET = frozenset(
    int(round(i * N_TILES / _N_GP)) for i in range(_N_GP)
)

_CACHE = {}


def _build_bass():
    import concourse.bacc as bacc
    import concourse.mybir as mybir
    import concourse.tile as tile

    f32 = mybir.dt.float32
    nc = bacc.Bacc("TRN2", target_bir_lowering=False, debug=False)

    xs = nc.dram_tensor("xs", [R, DW], f32, kind="ExternalInput")
    wbc = nc.dram_tensor("wbc", [P, T * DW], f32, kind="ExternalInput")
    binit = nc.dram_tensor("binit", [P, T * W7], f32, kind="ExternalInput")
    ys = nc.dram_tensor("ys", [R, G], f32, kind="ExternalOutput")

    xs_r = xs[:, :].rearrange("(n p t) d -> n p (t d)", p=P, t=T)
    ys_r = ys[:, :].rearrange("(n p t) g -> n p (t g)", p=P, t=T)

    with tile.TileContext(nc) as tc:
        with (
            tc.tile_pool(name="consts", bufs=1) as cpool,
            tc.tile_pool(name="wpsum", bufs=1, space="PSUM") as ppool,
            tc.tile_pool(name="xin", bufs=4) as xpool,
            tc.tile_pool(name="tmps", bufs=1) as tpool,
            tc.tile_pool(name="yout", bufs=4) as ypool,
        ):
            wt = cpool.tile([P, T * DW], f32, tag="wbc")
            nc.sync.dma_start(wt, wbc[:, :])
            wt4 = wt.rearrange("p (t g d) -> p t g d", t=T, g=G, d=D)

            # PSUM copy of the weights for the DVE muls (see _N_GP note).
            wtp = ppool.tile([P, T * DW], f32, tag="wpsum")
            nc.scalar.copy(wtp, wt)
            wtp4 = wtp.rearrange("p (t g d) -> p t g d", t=T, g=G, d=D)

            # Persistent tmp buffers: 7th column of each group pre-filled
            # with the bias; the muls only ever write columns 0..5.
            tmps = []
            for k in range(NTMP):
                tk = tpool.tile([P, T * W7], f32, tag=f"tmp{k}")
                nc.sync.dma_start(tk, binit[:, :])
                tmps.append(tk)

            for i in range(N_TILES):
                xt = xpool.tile([P, T * DW], f32, tag="x")
                nc.sync.dma_start(xt, xs_r[i])
                xt4 = xt.rearrange("p (t g d) -> p t g d", t=T, g=G, d=D)

                tmp = tmps[i % NTMP]
                tmp4 = tmp.rearrange("p (t g j) -> p t g j", t=T, g=G, j=D + 1)
                mul_out = tmp4[:, :, :, 0:D]
                if i in _GP_SET:
                    nc.gpsimd.tensor_tensor(
                        mul_out, xt4, wt4, mybir.AluOpType.mult
                    )
                else:
                    nc.vector.tensor_tensor(
                        mul_out, xt4, wtp4, mybir.AluOpType.mult
                    )

                yt = ypool.tile([P, T * G], f32, tag="y")
                tmp3 = tmp.rearrange("p (tg j) -> p tg j", j=D + 1)
                nc.vector.tensor_reduce(
                    yt, tmp3, mybir.AxisListType.X, mybir.AluOpType.add
                )
                nc.scalar.dma_start(ys_r[i], yt)

    nc.compile()
    return nc


def _get_bass():
    if "nc" not in _CACHE:
        _CACHE["nc"] = _build_bass()
    return _CACHE["nc"]


def _host_consts(W, b):
    # wbc[p, t*60 + g*6 + d] = W[g, d]
    wflat = np.ascontiguousarray(W, dtype=np.float32).reshape(DW)
    wbc = np.tile(wflat, (P, T)).astype(np.float32)
    # binit[p, t*70 + g*7 + j] = b[g] if j == 6 else 0
    brow = np.zeros((G, D + 1), dtype=np.float32)
    brow[:, D] = np.asarray(b, dtype=np.float32)
    binit = np.tile(brow.reshape(W7), (P, T)).astype(np.float32)
    return np.ascontiguousarray(wbc), np.ascontiguousarray(binit)


def _run(x, W, b, **spmd_kwargs):
    from concourse import bass_utils

    x = np.ascontiguousarray(x, dtype=np.float32)
    assert x.shape == (B_TOTAL, DW), x.shape
    wbc, binit = _host_consts(W, b)

    nc = _get_bass()
    in_maps = []
    for c in range(N_CORES):
        shard = x[c * R : (c + 1) * R]
        in_maps.append({"xs": shard, "wbc": wbc, "binit": binit})

    res = bass_utils.run_bass_kernel_spmd(
        nc, in_maps, core_ids=list(range(N_CORES)), **spmd_kwargs
    )
    y = np.concatenate([r["ys"] for r in res.results], axis=0)
    return y, res


def kernel(x, W, b):
    return _run(x, W, b)[0]
